# revision 8
# baseline (speedup 1.0000x reference)
"""Modulated deformable conv (DCNv2) on 8 trn2 NeuronCores, data-parallel over batch.

S-route algorithm (per core, one image), all-bf16 elementwise with f32 GEMM accum:
  per stripe (16 rows): offset/mask convs on PE (z,x stacked in 128 partitions)
    -> PE transpose -> OMT[j, plane, i] (planes: dy*9, dx*9, maskconv*9)
    -> tents TY/TX[e] = relu(1-|d-e|), e in -2..2; TYM = 2*sigmoid(mc)*TY (bf16);
       ring-flag partial sums via ACT accum_out (side 0:y+ 1:y- 2:x+ 3:x-).
  per band (= stripe, pipelined behind it):
    S_T[j,k,c,i] = sum_{ey,ex} (TYM*TX)[k,ey,ex,ij] * x[c, i+ky-1+ey, j+kx-1+ex]
    - inner 3x3 (ey,ex in {-1,0,1}) always: 27 kx-(ey,ex) term-pairs, ky folded
      via 3-plane APs, c-split DVE[0:CSPL) / gpsimd[CSPL:64)
    - outer ring (|ey|=2 or |ex|=2, no corners) gated per (band, side, kgroup)
      by tc.If on flags (weights are exactly zero unless |dy|>1 / |dx|>1 there)
    PE pair-transposes S_T -> [(c,k) chunks, (i,j)] psum-bf16, ACT evac,
    PE GEMM out[o,ij] = sum_{ck} w[o,ck] * SG[ck,ij], ACT evac, DMA out.
x source = host-pretransposed, pre-banded, 7-column-shifted copies (partition=j).
Out-of-image samples are zero (host zero-padding) matching torchvision semantics.
"""
import sys

sys.path.insert(0, "/opt/trn_rl_repo")

import numpy as np

import concourse.bass as bass
import concourse.mybir as mybir
import concourse.tile as tile
from concourse.bass_utils import run_bass_kernel_spmd

F32 = mybir.dt.float32
I32 = mybir.dt.int32
BF16 = mybir.dt.bfloat16
ALU = mybir.AluOpType
ACTF = mybir.ActivationFunctionType
AXT = mybir.AxisListType

H = W = 128
C = O = 64
KK = 9
PW = 134          # zx padded width/height, image at [3, 131)
R = 16            # output rows per band/stripe
NB = H // R       # 8 bands
XI = R + 6        # xb rows per band: image rows [i0-3, i0+R+3)
NSX = 7           # column shifts sx in [-3, 3]
NCORES = 8
CSPL = 36         # DVE handles c[0:CSPL], gpsimd c[CSPL:64) in the core MAC
CHUNKS = [(0, 1), (2, 3), (4, 5), (6, 7), (8,)]  # k -> GEMM contraction chunks
E3 = (-1, 0, 1)


def _fix_multiwait(nc, max_waits=1):
    """This walrus build accepts at most one sync-wait per instruction; hoist
    extras onto same-engine NoOps inserted just before."""
    import bass_rust

    ctr = 0
    for f in nc.m.functions:
        for bb in f.blocks:
            insts = bb.instructions

            def nwaits(i):
                si = i.sync_info
                return len(si.on_wait) if si is not None else 0

            if not any(nwaits(i) > max_waits for i in insts):
                continue
            out = []
            for inst in insts:
                si = inst.sync_info
                waits = list(si.on_wait) if si is not None else []
                if len(waits) > max_waits:
                    extra, keep = waits[:-max_waits], waits[-max_waits:]
                    for j in range(0, len(extra), max_waits):
                        ctr += 1
                        nop = mybir.InstNoOp(name=f"WFIX-{ctr}", ins=[], outs=[])
                        nop.engine = inst.engine
                        nop.sync_info = bass_rust.SyncInfo(
                            on_wait=extra[j : j + max_waits], on_update=[]
                        )
                        out.append(nop)
                    inst.sync_info = bass_rust.SyncInfo(
                        on_wait=keep, on_update=list(si.on_update)
                    )
                out.append(inst)
            bb.instructions = out


def _ap(t_ap, extra_off, dims):
    """Manual AP on a tile's backing tensor: dims = [[stride, n], ...] with the
    partition dim first (strides in elements)."""
    return bass.AP(tensor=t_ap.tensor, offset=t_ap.offset + extra_off, ap=dims)


def build_nc(cond_ring=True, fix_waits=True):
    nc = bass.Bass()
    zx = nc.dram_tensor("zx", [128, PW * PW], BF16, kind="ExternalInput")
    xb = nc.dram_tensor("xb", [128, NB * NSX * C * XI], BF16, kind="ExternalInput")
    wconv = nc.dram_tensor("wconv", [128, KK * 27], BF16, kind="ExternalInput")
    bias27 = nc.dram_tensor("bias27", [27, 1], F32, kind="ExternalInput")
    wg = nc.dram_tensor("wg", [128, 5 * O], BF16, kind="ExternalInput")
    ident = nc.dram_tensor("ident", [128, 128], BF16, kind="ExternalInput")
    consts = nc.dram_tensor("consts", [128, 8], F32, kind="ExternalInput")
    outO = nc.dram_tensor("outO", [O, H * W], F32, kind="ExternalOutput")

    with tile.TileContext(nc) as tc:
        with (
            tc.tile_pool(name="persist", bufs=1) as pp,
            tc.tile_pool(name="str1", bufs=2) as p1,
            tc.tile_pool(name="psc", bufs=2, space="PSUM") as pconv,
            tc.tile_pool(name="pst", bufs=1, space="PSUM") as ptr,
            tc.tile_pool(name="pfl", bufs=1, space="PSUM") as pfl,
            tc.tile_pool(name="bx", bufs=2) as pbx,
            tc.tile_pool(name="bst", bufs=2) as pbs,
            tc.tile_pool(name="bct", bufs=2) as pbc,
            tc.tile_pool(name="btmp", bufs=2) as pbt,
            tc.tile_pool(name="bsg", bufs=1) as psg,
            tc.tile_pool(name="ptp", bufs=1, space="PSUM") as ptp,
            tc.tile_pool(name="pgm", bufs=1, space="PSUM") as pgm,
        ):
            WC = pp.tile([128, KK, 27], BF16)
            WG = pp.tile([128, 5, O], BF16)
            IDT = pp.tile([128, 128], BF16)
            BIA = pp.tile([27, 1], F32)
            CST = pp.tile([128, 8], F32)  # cols 0-4: -e for e=-2..2; col5: 1.0
            OMT = pp.tile([128, 27, H], BF16)   # [j, plane, i]
            TX5 = pp.tile([128, 5, KK, H], BF16)
            TYM5 = pp.tile([128, 5, KK, H], BF16)
            ONES = pp.tile([128, 1], BF16)
            ONES32 = pp.tile([128, 1], F32)
            FBACC = pp.tile([128, NB, 12], F32)  # ring-flag partials, [band,(side,kg)]
            FLROW = pp.tile([1, NB * 12], I32)
            nc.sync.dma_start(WC[:], wconv.rearrange("p (t q) -> p t q", q=27))
            nc.sync.dma_start(WG[:], wg.rearrange("p (a b) -> p a b", b=O))
            nc.sync.dma_start(IDT[:], ident[:])
            nc.sync.dma_start(BIA[:], bias27[:])
            nc.sync.dma_start(CST[:], consts[:])
            nc.vector.memset(ONES[:], 1.0)
            nc.vector.memset(ONES32[:], 1.0)

            zx3 = zx.rearrange("p (a b) -> p a b", b=PW)

            # ================= per-stripe: conv -> OMT -> tents -> flags
            for s in range(NB):
                ZXS = p1.tile([128, 19, PW], BF16, tag="zxs")
                nc.sync.dma_start(
                    ZXS[:], zx3[:, s * 16 + 2 : s * 16 + 21, :]
                )
                OMS = p1.tile([27, 16, W], BF16, tag="oms")
                for q in range(4):
                    ps = pconv.tile([27, 512], F32, tag="convps")
                    for t in range(KK):
                        ty, tx = t // 3, t % 3
                        rhs = ZXS[:, 4 * q + ty : 4 * q + 4 + ty, 2 + tx : 2 + tx + W]
                        nc.tensor.matmul(
                            ps[:], WC[:, t, :], rhs,
                            start=(t == 0), stop=(t == KK - 1),
                        )
                    ps3 = ps[:].rearrange("p (a b) -> p a b", b=W)
                    nc.scalar.activation(
                        OMS[:, q * 4 : q * 4 + 4, :], ps3, ACTF.Identity,
                        bias=BIA[:, 0:1],
                    )
                pt = ptr.tile([128, 16, 28], BF16, tag="trps")
                for ii in range(16):
                    nc.tensor.transpose(
                        pt[:, ii, 0:27], OMS[:, ii, :], IDT[0:27, 0:27]
                    )
                # copy [j, (i,plane)] -> OMT [j, plane, i-slice]
                src = _ap(pt[:], 0, [[16 * 28, 128], [1, 27], [28, 16]])
                dst = _ap(OMT[:], s * 16, [[27 * H, 128], [H, 27], [1, 16]])
                nc.scalar.copy(dst, src)

                # tents for this stripe (i-slice s*16 ..)
                i0 = s * 16
                SIG = p1.tile([128, KK, R], BF16, tag="sig")
                nc.scalar.activation(
                    SIG[:], OMT[:, 18:27, i0 : i0 + R], ACTF.Sigmoid
                )
                for e in range(5):
                    nege = CST[:, e : e + 1]  # value -(e-2)
                    one = CST[:, 5:6]
                    edge = e in (0, 4)
                    side_y = 0 if e == 4 else 1
                    side_x = 2 if e == 4 else 3
                    TA = p1.tile([128, KK, R], BF16, tag=f"ta{e % 2}")
                    nc.scalar.activation(
                        TA[:], OMT[:, 0:9, i0 : i0 + R], ACTF.Abs, bias=nege
                    )
                    TB = p1.tile([128, KK, R], BF16, tag=f"tb{e % 2}")
                    if edge and cond_ring:
                        for kg in range(3):  # y-sides group k = [3kg, 3kg+3)
                            fac = _ap(
                                FBACC[:], s * 12 + side_y * 3 + kg,
                                [[NB * 12, 128], [1, 1]],
                            )
                            nc.scalar.activation(
                                TB[:, 3 * kg : 3 * kg + 3, :],
                                TA[:, 3 * kg : 3 * kg + 3, :],
                                ACTF.Relu, bias=one, scale=-1.0,
                                accum_out=fac,
                            )
                    else:
                        nc.scalar.activation(
                            TB[:], TA[:], ACTF.Relu, bias=one, scale=-1.0
                        )
                    # TYM = (TY * 2) * sigmoid
                    nc.vector.scalar_tensor_tensor(
                        out=TYM5[:, e, :, i0 : i0 + R], in0=TB[:], scalar=2.0,
                        in1=SIG[:], op0=ALU.mult, op1=ALU.mult,
                    )
                    TA2 = p1.tile([128, KK, R], BF16, tag=f"tc{e % 2}")
                    nc.scalar.activation(
                        TA2[:], OMT[:, 9:18, i0 : i0 + R], ACTF.Abs, bias=nege
                    )
                    if edge and cond_ring:
                        for kg in range(3):  # x-sides group k = kg::3
                            inap = _ap(
                                TA2[:], kg * R,
                                [[KK * R, 128], [3 * R, 3], [1, R]],
                            )
                            outap = _ap(
                                TX5[:], e * KK * H + kg * H + i0,
                                [[5 * KK * H, 128], [3 * H, 3], [1, R]],
                            )
                            fac = _ap(
                                FBACC[:], s * 12 + side_x * 3 + kg,
                                [[NB * 12, 128], [1, 1]],
                            )
                            nc.scalar.activation(
                                outap, inap, ACTF.Relu, bias=one, scale=-1.0,
                                accum_out=fac,
                            )
                    else:
                        nc.scalar.activation(
                            TX5[:, e, :, i0 : i0 + R], TA2[:], ACTF.Relu,
                            bias=one, scale=-1.0,
                        )

            # ================= per-band main loop
            flregs = nc.vector.alloc_register("flgd")
            ct_tiles = {}

            def build_ct(ib):
                CT = pbc.tile([128, KK, 5, 5, R], BF16, tag="ct")
                ct_tiles[ib] = CT
                ctp = CT[:]
                i0 = ib * R
                for e in range(5):
                    out = _ap(
                        ctp, e * 5 * R,
                        [[KK * 25 * R, 128], [25 * R, KK], [R, 5], [1, R]],
                    )
                    in0 = _ap(
                        TYM5[:], e * KK * H + i0,
                        [[5 * KK * H, 128], [H, KK], [0, 5], [1, R]],
                    )
                    in1 = _ap(
                        TX5[:], i0,
                        [[5 * KK * H, 128], [H, KK], [KK * H, 5], [1, R]],
                    )
                    nc.vector.tensor_tensor(out=out, in0=in0, in1=in1, op=ALU.mult)

            build_ct(0)
            for ib in range(NB):
                i0 = ib * R
                XB = pbx.tile([128, NSX, C, XI], BF16, tag="xb")
                nc.sync.dma_start(
                    XB[:],
                    xb[:, ib * NSX * C * XI : (ib + 1) * NSX * C * XI].rearrange(
                        "p (s c i) -> p s c i", c=C, i=XI
                    ),
                )
                if cond_ring:
                    # flags: sum over j of FBACC band cols; >0 -> fire
                    pf = pfl.tile([1, 12], F32, tag="fl")
                    nc.tensor.matmul(
                        pf[:], ONES32[:], FBACC[:, ib, :], start=True, stop=True
                    )
                    nc.vector.tensor_scalar(
                        FLROW[0:1, ib * 12 : ib * 12 + 12], pf[:], 0.0, None,
                        op0=ALU.is_gt,
                    )

                ctp = ct_tiles[ib][:]
                ST = pbs.tile([128, KK, C, R], BF16, tag="st")
                stp = ST[:]
                xbp = XB[:]

                def mac(eng, kx, ey, ex, c0, cn, first, tmp_tag):
                    sx = kx - 1 + ex
                    xin = _ap(
                        xbp,
                        (sx + 3) * C * XI + c0 * XI + (ey + 2),
                        [[NSX * C * XI, 128], [1, 3], [XI, cn], [1, R]],
                    )
                    ctin = _ap(
                        ctp,
                        kx * 25 * R + (ey + 2) * 5 * R + (ex + 2) * R,
                        [[KK * 25 * R, 128], [3 * 25 * R, 3], [0, cn], [1, R]],
                    )
                    acc = _ap(
                        stp,
                        kx * C * R + c0 * R,
                        [[KK * C * R, 128], [3 * C * R, 3], [R, cn], [1, R]],
                    )
                    if first:
                        eng.tensor_tensor(out=acc, in0=xin, in1=ctin, op=ALU.mult)
                    else:
                        tmp = pbt.tile([128, 3, cn, R], BF16, tag=tmp_tag)
                        eng.tensor_tensor(
                            out=tmp[:], in0=xin, in1=ctin, op=ALU.mult
                        )
                        eng.tensor_tensor(
                            out=acc, in0=acc, in1=tmp[:], op=ALU.add
                        )

                # core 3x3
                for ti, (ey, ex) in enumerate([(a, b) for a in E3 for b in E3]):
                    if ti == 4 and ib + 1 < NB:
                        build_ct(ib + 1)  # prefetch next band's tent products
                    for kx in range(3):
                        mac(nc.vector, kx, ey, ex, 0, CSPL, ti == 0, "tva")
                        mac(nc.gpsimd, kx, ey, ex, CSPL, C - CSPL, ti == 0, "tvb")

                # ring terms (exact tail; weights zero unless |d|>1 present)
                def ring_y(eng, side, ky, c0, cn, tag):
                    ey = 2 if side == 0 else -2
                    for ex in E3:
                        sx0 = -1 + ex  # kx=0
                        xin = _ap(
                            xbp,
                            (sx0 + 3) * C * XI + c0 * XI + (ky + ey + 2),
                            [[NSX * C * XI, 128], [C * XI, 3], [XI, cn],
                             [1, R]],
                        )
                        ctin = _ap(
                            ctp,
                            (3 * ky) * 25 * R + (ey + 2) * 5 * R + (ex + 2) * R,
                            [[KK * 25 * R, 128], [25 * R, 3], [0, cn], [1, R]],
                        )
                        acc = _ap(
                            stp,
                            (3 * ky) * C * R + c0 * R,
                            [[KK * C * R, 128], [C * R, 3], [R, cn], [1, R]],
                        )
                        tmp = pbt.tile([128, 3, cn, R], BF16, tag=tag)
                        eng.tensor_tensor(out=tmp[:], in0=xin, in1=ctin,
                                          op=ALU.mult)
                        eng.tensor_tensor(out=acc, in0=acc, in1=tmp[:],
                                          op=ALU.add)

                def ring_cell(side, kg):
                    if side < 2:  # y-ring: ey=+-2, ky=kg fixed, kx folded
                        ring_y(nc.vector, side, kg, 0, C, "tvr")
                    else:  # x-ring: ex=+-2, kx=kg fixed, ky folded
                        ex = 2 if side == 2 else -2
                        for ey in E3:
                            mac(nc.vector, kg, ey, ex, 0, C, False, "tvr")

                for side in range(4):
                    for kg in range(3):
                        if cond_ring:
                            fi = ib * 12 + side * 3 + kg
                            nc.reg_load(flregs, FLROW[0:1, fi : fi + 1])
                            with tc._internal_If_cmp(flregs, 0, "IS_GT"):
                                ring_cell(side, kg)
                        else:
                            ring_cell(side, kg)

                # transpose to [(c,k)-chunks, (i,j)] and evacuate
                SG = psg.tile([128, 5, R, W], BF16, tag="sg")
                for p, ks in enumerate(CHUNKS):
                    m = len(ks) * C
                    pt = ptp.tile([128, R * W], BF16, tag="tp")
                    for ii in range(R):
                        lhsT = _ap(
                            stp, ks[0] * C * R + ii,
                            [[KK * C * R, 128], [C * R, len(ks)], [R, C]],
                        )
                        nc.tensor.transpose(
                            pt[0:m, ii * W : (ii + 1) * W], lhsT, IDT[:]
                        )
                    nc.scalar.copy(
                        SG[0:m, p].rearrange("p a b -> p (a b)"), pt[0:m, :]
                    )

                # GEMM in half-bands (512-col pieces: one PSUM bank per group)
                for h in range(2):
                    hw0 = h * (R * W // 2)
                    pg = pgm.tile([O, R * W // 2], F32, tag="gm")
                    for n0 in range(0, R * W // 2, 512):
                        for p, ks in enumerate(CHUNKS):
                            m = len(ks) * C
                            rhs = _ap(
                                SG[:], p * R * W + hw0 + n0,
                                [[5 * R * W, m], [1, 512]],
                            )
                            nc.tensor.matmul(
                                pg[:, n0 : n0 + 512], WG[0:m, p, :], rhs,
                                start=(p == 0), stop=(p == len(CHUNKS) - 1),
                            )
                    OUTS = psg.tile([O, R * W // 2], F32, tag="outs")
                    nc.scalar.copy(OUTS[:], pg[:])
                    nc.sync.dma_start(
                        outO[:, i0 * W + hw0 : i0 * W + hw0 + R * W // 2],
                        OUTS[:],
                    )

    if fix_waits:
        _fix_multiwait(nc)
    return nc


def make_consts(w_off, b_off, w_mod, b_mod, w_reg):
    wconv = np.zeros((128, KK, 27), np.float32)
    for t in range(KK):
        ty, tx = t // 3, t % 3
        wconv[0:64, t, 0:18] = w_off[:, :, ty, tx].T     # z half -> offsets
        wconv[64:128, t, 18:27] = w_mod[:, :, ty, tx].T  # x half -> mask
    # reorder offset channels so planes are [dy*9, dx*9, mask*9]
    perm = list(range(0, 18, 2)) + list(range(1, 18, 2)) + list(range(18, 27))
    wconv = wconv[:, :, perm].reshape(128, KK * 27)
    bias27 = np.concatenate([b_off[perm[:18]], b_mod]).reshape(27, 1).astype(
        np.float32
    )
    # GEMM chunk weights: wg[c + 64*q, p, o] = w_reg[o, c, k=2p+q]
    w3 = w_reg.reshape(O, C, KK)
    wgm = np.zeros((128, 5, O), np.float32)
    for p, ks in enumerate(CHUNKS):
        for q, k in enumerate(ks):
            wgm[q * 64 : q * 64 + 64, p, :] = w3[:, :, k].T
    ident = np.eye(128, dtype=np.float32)
    consts = np.zeros((128, 8), np.float32)
    consts[:, 0:5] = np.array([2.0, 1.0, 0.0, -1.0, -2.0], np.float32)
    consts[:, 5] = 1.0
    return wconv, bias27, wgm.reshape(128, 5 * O), ident, consts


def make_zx(z_img, x_img, bf):
    zxp = np.zeros((128, PW, PW), bf)
    zxp[0:64, 3 : 3 + H, 3 : 3 + W] = z_img.astype(bf)
    zxp[64:128, 3 : 3 + H, 3 : 3 + W] = x_img.astype(bf)
    return zxp.reshape(128, PW * PW)


def make_xb(x_img, bf):
    # xb[j, band, sx5, c, il] = xpad[c, i0+il, j+sx5] ; pad 3 all around
    xp = np.zeros((C, H + 6, W + 6), bf)
    xp[:, 3 : 3 + H, 3 : 3 + W] = x_img.astype(bf)
    out = np.empty((128, NB, NSX, C, XI), bf)
    for s5 in range(NSX):
        tr = xp[:, :, s5 : s5 + 128].transpose(2, 0, 1)  # [j, c, 134]
        for ib in range(NB):
            out[:, ib, s5] = tr[:, :, ib * R : ib * R + XI]
    return out.reshape(128, NB * NSX * C * XI)


_NC_CACHE = None


def _get_nc():
    global _NC_CACHE
    if _NC_CACHE is None:
        _NC_CACHE = build_nc()
    return _NC_CACHE


def _make_in_maps(inp):
    import ml_dtypes

    bf = ml_dtypes.bfloat16
    x = np.asarray(inp["x"], np.float32)
    z = np.asarray(inp["z"], np.float32)
    wconv, bias27, wgm, ident, consts = make_consts(
        np.asarray(inp["w_off"], np.float32), np.asarray(inp["b_off"], np.float32),
        np.asarray(inp["w_mod"], np.float32), np.asarray(inp["b_mod"], np.float32),
        np.asarray(inp["w_reg"], np.float32),
    )
    wconv_bf = wconv.astype(bf)
    wg_bf = wgm.astype(bf)
    ident_bf = ident.astype(bf)
    in_maps = []
    for b in range(x.shape[0]):
        in_maps.append(
            dict(
                zx=make_zx(z[b], x[b], bf),
                xb=make_xb(x[b], bf),
                wconv=wconv_bf,
                bias27=bias27,
                wg=wg_bf,
                ident=ident_bf,
                consts=consts,
            )
        )
    return in_maps


def kernel(x, z, w_off, b_off, w_mod, b_mod, w_reg):
    in_maps = _make_in_maps(
        dict(x=x, z=z, w_off=w_off, b_off=b_off, w_mod=w_mod, b_mod=b_mod,
             w_reg=w_reg)
    )
    nc = _get_nc()
    res = run_bass_kernel_spmd(nc, in_maps, list(range(NCORES)))
    out = np.stack(
        [res.results[b]["outO"].reshape(O, H, W) for b in range(len(in_maps))]
    ).astype(np.float32)
    return out
